# revision 1
# baseline (speedup 1.0000x reference)
"""AffineCoupling TRN2 kernel.

Computes, for z [4_000_000, 16] fp32:
    zl = z[:, :8]; zr = z[:, 8:]
    log_s = MLP_logs(zl); b = MLP_b(zl)        (5 layers, LeakyReLU(0.01) between)
    out = concat([zl, exp(log_s) * zr + b], axis=1)

Strategy (pure data parallel over 8 NeuronCores):
 - Each core gets a 507,904-row slice (slices overlap slightly to cover 4M).
 - On-chip layout: "nat" tile [128, 2048] holds 16,384 rows, 8 rows/partition
   per 128-col sub-tile: nat[p, s*128 + g*16 + f] = z[base + s*1024 + p*8 + g, f].
   HBM reads/writes are 512B-contiguous per partition.
 - PE transposes each [128,128] sub-tile to feature-major (X0[g*16+f, col]).
 - MLP = 5 bf16 matmuls per 4096-row chunk against block-diagonal augmented
   weights (both branches fused; fp32->bf16 casts ride the PSUM->SBUF copies);
   LeakyReLU = ACT Prelu(alpha=0.01) reading the fp32 PSUM, writing bf16,
   with the layer bias applied via the activation's per-partition bias operand.
 - L5 writes fp32 [log_s (parts 0:64, g*8+o) ; b (parts 64:128)]; Exp and a
   bias-add assemble eb = [e; b] in fp32, which is PE-transposed back to
   batch-major and combined with zr in the nat tile in place: yr = e * zr + b.
 - The nat tile (zl untouched, zr overwritten with yr) is DMA'd back out.
"""
import os
import sys

sys.path.insert(0, "/opt/trn_rl_repo")
if "/root/.axon_site/_ro/trn_rl_repo" not in sys.path:
    sys.path.append("/root/.axon_site/_ro/trn_rl_repo")

import numpy as np

import concourse.bacc as bacc
import concourse.bass as bass
import concourse.tile as tile
from concourse import mybir
from concourse.bass import _add_dep_helper
from concourse.bass_utils import run_bass_kernel_spmd

FP = mybir.dt.float32
BF = mybir.dt.bfloat16

N_CORES = 8
BATCH = 4_000_000
ROWS_PER_MACRO = 16_384            # [128, 2048] nat tile
MACROS = 31
R = ROWS_PER_MACRO * MACROS        # 507,904 rows per core
PAD_ROWS = ROWS_PER_MACRO          # guard band: writes never touch tensor tail
SUBTILES = 16                      # per macro, 1024 rows each
CHUNKS = 4                         # per macro, 4096 rows each (4 sub-tiles)
NAT_BUFS = 3

STEP = 498_688
# cores 0..6 tile forward; the last core is pinned to cover the batch tail
STARTS = [c * STEP for c in range(N_CORES - 1)] + [BATCH - R]

C_BIAS = 128                       # fp32 consts: identity + 5 bias columns
C_TOTAL = 133

LAST_RESULTS = None


def _build_consts(ws_logs, bs_logs, ws_b, bs_b):
    import ml_dtypes

    ws_logs = [np.asarray(w, np.float32) for w in ws_logs]
    bs_logs = [np.asarray(b, np.float32) for b in bs_logs]
    ws_b = [np.asarray(w, np.float32) for w in ws_b]
    bs_b = [np.asarray(b, np.float32) for b in bs_b]

    consts = np.zeros((128, C_TOTAL), np.float32)
    consts[:, 0:128] = np.eye(128, dtype=np.float32)
    for k in range(4):
        cat = np.concatenate([bs_logs[k], bs_b[k]])    # [16]
        consts[:, C_BIAS + k] = np.tile(cat, 8)
    consts[:, C_BIAS + 4] = np.concatenate(
        [np.tile(bs_logs[4], 8), np.tile(bs_b[4], 8)]
    )

    # bf16 stationary matrices, lhsT k at cols [k*128, (k+1)*128)
    wmat = np.zeros((128, 5 * 128), np.float32)
    # L1: input rows g*16+i (i<8: zl features), output cols g*16+o_cat
    w1cat = np.vstack([ws_logs[0], ws_b[0]])           # [16, 8]
    for g in range(8):
        wmat[g * 16:g * 16 + 8, g * 16:(g + 1) * 16] = w1cat.T
    for k in (1, 2, 3):
        wk = np.zeros((16, 16), np.float32)
        wk[0:8, 0:8] = ws_logs[k]
        wk[8:16, 8:16] = ws_b[k]
        for g in range(8):
            wmat[g * 16:(g + 1) * 16, k * 128 + g * 16:k * 128 + (g + 1) * 16] = wk.T
    for g in range(8):
        wmat[g * 16:g * 16 + 8, 4 * 128 + g * 8:4 * 128 + (g + 1) * 8] = ws_logs[4].T
        wmat[g * 16 + 8:(g + 1) * 16,
             4 * 128 + 64 + g * 8:4 * 128 + 64 + (g + 1) * 8] = ws_b[4].T
    wmat = np.concatenate([wmat, np.eye(128, dtype=np.float32)], axis=1)
    wmat_bf = wmat.astype(ml_dtypes.bfloat16)
    return consts, wmat_bf


def _free_ap(t, offset, dims):
    """AP over tile t with the tile's partition dim, explicit free dims
    [[step, count], ...] and an element offset into the free space."""
    return bass.AP(tensor=t.tensor, offset=t.offset + offset, ap=[t.ap[0]] + dims)


def _build_nc():
    nc = bacc.Bacc()
    z_d = nc.declare_dram_parameter("z", [R + PAD_ROWS, 16], FP, isOutput=False)
    c_d = nc.declare_dram_parameter("consts", [128, C_TOTAL], FP, isOutput=False)
    w_d = nc.declare_dram_parameter("wmat", [128, 6 * 128], BF, isOutput=False)
    o_d = nc.declare_dram_parameter("out", [R + PAD_ROWS, 16], FP, isOutput=True)

    with tile.TileContext(nc) as tc:
        with (
            tc.tile_pool(name="consts", bufs=1) as cp,
            tc.tile_pool(name="nat", bufs=4) as natp,
            tc.tile_pool(name="sb", bufs=4) as sbp,
            tc.tile_pool(name="ps", bufs=2, space="PSUM") as psp,
            tc.tile_pool(name="hps", bufs=3, space="PSUM") as hpsp,
        ):
            consts = cp.tile([128, C_TOTAL], FP)
            nc.sync.dma_start(out=consts, in_=c_d[:, :])
            wmat = cp.tile([128, 6 * 128], BF)
            nc.sync.dma_start(out=wmat, in_=w_d[:, :])
            ident = consts[:, 0:128]
            identbf = wmat[:, 5 * 128:6 * 128]
            lhsT = [wmat[:, k * 128:(k + 1) * 128] for k in range(5)]
            biases = [consts[:, C_BIAS + k:C_BIAS + k + 1] for k in range(5)]

            # warm up each engine's vector clock on the const DMAs
            wu_ps = psp.tile([128, 128], FP, tag="tp")
            nc.tensor.matmul(wu_ps, ident, ident, start=True, stop=True)
            wu_ps2 = hpsp.tile([64, 64], FP, tag="hp")
            nc.tensor.matmul(wu_ps2, lhsT[0][:, 0:64], wmat[:, 0:64],
                             start=True, stop=True)
            wu1 = sbp.tile([128, 1], FP, tag="wu")
            nc.scalar.copy(out=wu1, in_=biases[0])
            wu2 = sbp.tile([128, 1], FP, tag="wu")
            nc.vector.tensor_copy(out=wu2, in_=biases[0])
            wu3 = sbp.tile([128, 1], FP, tag="wu")
            nc.gpsimd.tensor_copy(out=wu3, in_=biases[0])

            tail_dmas = []
            for m in range(MACROS):
                r0 = m * ROWS_PER_MACRO
                nat = natp.tile([128, 2048], FP, tag="nat")
                nc.sync.dma_start(
                    out=nat.rearrange("p (s g f) -> p s g f", s=SUBTILES, g=8, f=16),
                    in_=z_d[r0:r0 + ROWS_PER_MACRO, :].rearrange(
                        "(s p g) f -> p s g f", s=SUBTILES, p=128, g=8
                    ),
                )

                natbfs = []
                for k in range(CHUNKS):
                    natbf = sbp.tile([128, 512], BF, tag="natbf", bufs=12)
                    nc.gpsimd.tensor_copy(
                        out=natbf, in_=nat[:, k * 512:(k + 1) * 512])
                    natbfs.append(natbf)

                for j in range(CHUNKS // 2):          # chunk pairs
                    x0s = []
                    for c in range(2):                # per-chunk transposes + cast
                        k = 2 * j + c
                        x0ps = psp.tile([128, 512], FP, tag="tp")
                        natbf = natbfs[k]
                        for t in range(4):
                            nc.tensor.matmul(
                                x0ps[:, t * 128:(t + 1) * 128],
                                natbf[:, t * 128:(t + 1) * 128],
                                identbf,
                                start=True, stop=True,
                            )
                        x0 = sbp.tile([128, 512], BF, tag="x0", bufs=8)
                        nc.vector.tensor_copy(out=x0, in_=x0ps)
                        x0s.append(x0)

                    # ---- MLP: bf16 matmul pairs -> [128,1024] Prelu
                    h = x0s
                    for layer in range(4):
                        hp = hpsp.tile([128, 1024], FP, tag="hp")
                        for c in range(2):
                            nc.tensor.matmul(hp[:, c * 512:(c + 1) * 512],
                                             lhsT[layer], h[c],
                                             start=True, stop=True)
                        hb = sbp.tile([128, 1024], BF, tag="h", bufs=8)
                        nc.scalar.activation(
                            out=hb, in_=hp,
                            func=mybir.ActivationFunctionType.Prelu,
                            bias=biases[layer], scale=1.0, alpha=0.01,
                        )
                        h = [hb[:, 0:512], hb[:, 512:1024]]
                    hp5 = hpsp.tile([128, 1024], FP, tag="hp")
                    for c in range(2):
                        nc.tensor.matmul(hp5[:, c * 512:(c + 1) * 512],
                                         lhsT[4], h[c], start=True, stop=True)

                    # ---- eb = [exp(log_s + b5L) ; b + b5R]   (fp32, both chunks)
                    eb = sbp.tile([128, 1024], FP, tag="eb", bufs=6)
                    nc.scalar.activation(
                        out=eb[0:64, :], in_=hp5[0:64, :],
                        func=mybir.ActivationFunctionType.Exp,
                        bias=biases[4][0:64, :], scale=1.0,
                    )
                    nc.vector.tensor_scalar_add(
                        out=eb[64:128, :], in0=hp5[64:128, :],
                        scalar1=biases[4][64:128, :],
                    )

                    # ---- per chunk: transpose back + yr = e*zr + b in place
                    for c in range(2):
                        k = 2 * j + c
                        ebT = psp.tile([128, 512], FP, tag="tp")
                        for t in range(4):
                            nc.tensor.transpose(
                                ebT[:, t * 128:(t + 1) * 128],
                                eb[:, c * 512 + t * 128:c * 512 + (t + 1) * 128],
                                ident,
                            )
                        e_ap = _free_ap(ebT, 0, [[128, 4], [8, 8], [1, 8]])
                        b_ap = _free_ap(ebT, 64, [[128, 4], [8, 8], [1, 8]])
                        zr_ap = _free_ap(nat, k * 512 + 8,
                                         [[128, 4], [16, 8], [1, 8]])
                        tmp = sbp.tile([128, 256], FP, tag="tmp", bufs=8)
                        tmp_ap = _free_ap(tmp, 0, [[64, 4], [8, 8], [1, 8]])
                        nc.vector.tensor_mul(out=tmp_ap, in0=e_ap, in1=zr_ap)
                        nc.vector.tensor_add(out=zr_ap, in0=tmp_ap, in1=b_ap)

                out_dma = nc.sync.dma_start(
                    out=o_d[r0:r0 + ROWS_PER_MACRO, :].rearrange(
                        "(s p g) f -> p s g f", s=SUBTILES, p=128, g=8
                    ),
                    in_=nat.rearrange("p (s g f) -> p s g f", s=SUBTILES, g=8, f=16),
                )
                if m >= MACROS - NAT_BUFS:
                    tail_dmas.append(out_dma)

            flush = sbp.tile([128, 1], FP, tag="wu")
            fl = nc.vector.tensor_copy(out=flush, in_=biases[0])
            for dma in tail_dmas:
                _add_dep_helper(fl.ins, dma.ins, sync=True,
                                reason="drain tail out-DMAs before kernel end")

    nc.finalize()
    return nc


_NC_CACHE = None


def kernel(z, ws_logs, bs_logs, ws_b, bs_b):
    global _NC_CACHE, LAST_RESULTS
    z = np.asarray(z, np.float32)
    assert z.shape == (BATCH, 16)
    consts, wmat_bf = _build_consts(ws_logs, bs_logs, ws_b, bs_b)

    if _NC_CACHE is None:
        _NC_CACHE = _build_nc()
    nc = _NC_CACHE

    in_maps = []
    for s in STARTS:
        zp = np.zeros((R + PAD_ROWS, 16), np.float32)
        zp[:R] = z[s:s + R]
        in_maps.append({"z": zp, "consts": consts, "wmat": wmat_bf})
    trace = bool(os.environ.get("AFFINE_TRACE"))
    res = run_bass_kernel_spmd(nc, in_maps, core_ids=list(range(N_CORES)), trace=trace)
    LAST_RESULTS = res

    out = np.empty((BATCH, 16), np.float32)
    for c in range(N_CORES):
        out[STARTS[c]:STARTS[c] + R] = res.results[c]["out"][:R]
    return out



# revision 3
# speedup vs baseline: 1.4150x; 1.4150x over previous
"""AffineCoupling TRN2 kernel (v6).

Computes, for z [4_000_000, 16] fp32:
    zl = z[:, :8]; zr = z[:, 8:]
    log_s = MLP_logs(zl); b = MLP_b(zl)        (5 layers, LeakyReLU(0.01) between)
    out = concat([zl, yr]), yr = exp(log_s) * zr + b

Strategy (pure data parallel over 8 NeuronCores, ~508k rows each):
 - Contiguous DMA: core slice split into 31 macros of 16384 rows. natbf
   [128, 2048] bf16 holds 128 rows/partition (nat[p, c*16+f] = row p*128+c),
   loaded by ONE SWDGE cast-DMA (fp32 HBM -> bf16 SBUF, 8KB contiguous HBM
   per partition) and stored back by one SWDGE cast-DMA (bf16 -> fp32).
   The whole pipeline is bf16 (zl passthrough in bf16: ~1.3e-3 rel err,
   tolerance is 2e-2).
 - fwdT: 16 PE transpose-mode ops [128,128] -> x0ps bf16 PSUM (feature-major
   X layout: partition g*16+f, 8 groups of 16 feats); DVE 2x copy -> x0.
 - MLP: both branches fused in 16-wide groups (block-diagonal bf16 lhsT,
   same wmat as before); per layer 4 MMs N=512 -> h fp32 PSUM [128, 2048],
   one ACT Prelu (bias via per-partition operand) -> bf16 SBUF.
 - L5 -> hp5 [128, 2048] fp32 (e at partitions 0:64 as g*8+o, b at 64:128);
   ACT Exp (+bias) and DVE tensor_scalar_add assemble eb bf16.
 - backT: 16 transpose-mode ops -> ebT bf16 PSUM; combine in place:
   natbf_zr = e*zr + b via 2x (mul into tmp, add back), u-batched APs.
 - PSUM: x0ps(2) + h(4) + ebT(2) = 8 banks, single-buffered per tag;
   cross-macro overlap comes from fwdT/backT of adjacent macros.
"""
import os
import sys

sys.path.insert(0, "/opt/trn_rl_repo")
if "/root/.axon_site/_ro/trn_rl_repo" not in sys.path:
    sys.path.append("/root/.axon_site/_ro/trn_rl_repo")

import numpy as np

import concourse.bacc as bacc
import concourse.bass as bass
import concourse.tile as tile
from concourse import mybir
from concourse.bass import _add_dep_helper
from concourse.bass_utils import run_bass_kernel_spmd

FP = mybir.dt.float32
BF = mybir.dt.bfloat16

N_CORES = 8
BATCH = 4_000_000
ROWS_PER_MACRO = 16_384            # [128, 2048] bf16 nat tile, 128 rows/part
MACROS = 31
R = ROWS_PER_MACRO * MACROS        # 507,904 rows per core
PAD_ROWS = ROWS_PER_MACRO
CHUNKS = 4                         # 4096 rows each

STEP = 498_688
STARTS = [c * STEP for c in range(N_CORES - 1)] + [BATCH - R]

C_BIAS = 128
C_TOTAL = 133

LAST_RESULTS = None


def _build_consts(ws_logs, bs_logs, ws_b, bs_b):
    import ml_dtypes

    ws_logs = [np.asarray(w, np.float32) for w in ws_logs]
    bs_logs = [np.asarray(b, np.float32) for b in bs_logs]
    ws_b = [np.asarray(w, np.float32) for w in ws_b]
    bs_b = [np.asarray(b, np.float32) for b in bs_b]

    consts = np.zeros((128, C_TOTAL), np.float32)
    consts[:, 0:128] = np.eye(128, dtype=np.float32)
    for k in range(4):
        cat = np.concatenate([bs_logs[k], bs_b[k]])    # [16]
        consts[:, C_BIAS + k] = np.tile(cat, 8)
    consts[:, C_BIAS + 4] = np.concatenate(
        [np.tile(bs_logs[4], 8), np.tile(bs_b[4], 8)]
    )

    # bf16 stationary matrices, lhsT k at cols [k*128, (k+1)*128)
    wmat = np.zeros((128, 5 * 128), np.float32)
    w1cat = np.vstack([ws_logs[0], ws_b[0]])           # [16, 8]
    for g in range(8):
        wmat[g * 16:g * 16 + 8, g * 16:(g + 1) * 16] = w1cat.T
    for k in (1, 2, 3):
        wk = np.zeros((16, 16), np.float32)
        wk[0:8, 0:8] = ws_logs[k]
        wk[8:16, 8:16] = ws_b[k]
        for g in range(8):
            wmat[g * 16:(g + 1) * 16, k * 128 + g * 16:k * 128 + (g + 1) * 16] = wk.T
    for g in range(8):
        wmat[g * 16:g * 16 + 8, 4 * 128 + g * 8:4 * 128 + (g + 1) * 8] = ws_logs[4].T
        wmat[g * 16 + 8:(g + 1) * 16,
             4 * 128 + 64 + g * 8:4 * 128 + 64 + (g + 1) * 8] = ws_b[4].T
    wmat = np.concatenate([wmat, np.eye(128, dtype=np.float32)], axis=1)
    wmat_bf = wmat.astype(ml_dtypes.bfloat16)
    return consts, wmat_bf


def _ap(t, offset, dims):
    return bass.AP(tensor=t.tensor, offset=t.offset + offset, ap=[t.ap[0]] + dims)


def _build_nc():
    nc = bacc.Bacc()
    z_d = nc.declare_dram_parameter("z", [R + PAD_ROWS, 16], FP, isOutput=False)
    c_d = nc.declare_dram_parameter("consts", [128, C_TOTAL], FP, isOutput=False)
    w_d = nc.declare_dram_parameter("wmat", [128, 6 * 128], BF, isOutput=False)
    o_d = nc.declare_dram_parameter("out", [R + PAD_ROWS, 16], FP, isOutput=True)

    with tile.TileContext(nc) as tc:
        with (
            tc.tile_pool(name="consts", bufs=1) as cp,
            tc.tile_pool(name="nat", bufs=1) as natp,
            tc.tile_pool(name="sb", bufs=1) as sbp,
            tc.tile_pool(name="ps", bufs=1, space="PSUM") as psp,
        ):
            consts = cp.tile([128, C_TOTAL], FP)
            nc.sync.dma_start(out=consts, in_=c_d[:, :])
            wmat = cp.tile([128, 6 * 128], BF)
            nc.sync.dma_start(out=wmat, in_=w_d[:, :])
            identbf = wmat[:, 5 * 128:6 * 128]
            lhsT = [wmat[:, k * 128:(k + 1) * 128] for k in range(5)]
            biases = [consts[:, C_BIAS + k:C_BIAS + k + 1] for k in range(5)]

            # warm-up scalar/vector vector clocks on the const DMAs
            wu1 = sbp.tile([128, 1], FP, tag="wu")
            nc.scalar.copy(out=wu1, in_=biases[0])
            wu2 = sbp.tile([128, 1], FP, tag="wu")
            nc.vector.tensor_copy(out=wu2, in_=biases[0])

            tail_dmas = []
            for m in range(MACROS):
                r0 = m * ROWS_PER_MACRO
                natbf = natp.tile([128, 2048], BF, tag="nat", bufs=3)
                nc.gpsimd.dma_start(
                    out=natbf.rearrange("p (c f) -> p c f", c=128, f=16),
                    in_=z_d[r0:r0 + ROWS_PER_MACRO, :].rearrange(
                        "(p c) f -> p c f", p=128, c=128
                    ),
                )

                # ---- fwd transposes -> x0ps (bf16 PSUM) -> x0 (bf16 SBUF)
                x0ps = psp.tile([128, 2048], BF, tag="x0ps", bufs=1)
                for u in range(16):
                    nc.tensor.transpose(
                        x0ps[:, u * 128:(u + 1) * 128],
                        natbf[:, u * 128:(u + 1) * 128],
                        identbf,
                    )
                x0 = sbp.tile([128, 2048], BF, tag="x0", bufs=2)
                nc.vector.tensor_copy(out=x0, in_=x0ps)

                # ---- MLP layers 1..4
                h = x0
                for layer in range(4):
                    hps = psp.tile([128, 2048], FP, tag="h", bufs=1)
                    for n in range(CHUNKS):
                        nc.tensor.matmul(
                            hps[:, n * 512:(n + 1) * 512],
                            lhsT[layer], h[:, n * 512:(n + 1) * 512],
                            start=True, stop=True,
                        )
                    hb = sbp.tile([128, 2048], BF, tag=f"h{layer}", bufs=2)
                    nc.scalar.activation(
                        out=hb, in_=hps,
                        func=mybir.ActivationFunctionType.Prelu,
                        bias=biases[layer], scale=1.0, alpha=0.01,
                    )
                    h = hb

                # ---- L5 -> hp5; eb = [exp(log_s+b5e) ; b+b5b]  (bf16)
                hp5 = psp.tile([128, 2048], FP, tag="h", bufs=1)
                for n in range(CHUNKS):
                    nc.tensor.matmul(
                        hp5[:, n * 512:(n + 1) * 512],
                        lhsT[4], h[:, n * 512:(n + 1) * 512],
                        start=True, stop=True,
                    )
                eb = sbp.tile([128, 2048], BF, tag="eb", bufs=2)
                nc.scalar.activation(
                    out=eb[0:64, :], in_=hp5[0:64, :],
                    func=mybir.ActivationFunctionType.Exp,
                    bias=biases[4][0:64, :], scale=1.0,
                )
                nc.vector.tensor_scalar_add(
                    out=eb[64:128, :], in0=hp5[64:128, :],
                    scalar1=biases[4][64:128, :],
                )

                # ---- back transposes -> ebT (bf16 PSUM)
                ebT = psp.tile([128, 2048], BF, tag="ebT", bufs=1)
                for u in range(16):
                    nc.tensor.transpose(
                        ebT[:, u * 128:(u + 1) * 128],
                        eb[:, u * 128:(u + 1) * 128],
                        identbf,
                    )

                # ---- combine in place: natbf_zr = e*zr + b  (u-batched, 2 halves)
                for half in range(2):
                    off = half * 1024
                    e_ap = _ap(ebT, off, [[128, 8], [8, 8], [1, 8]])
                    b_ap = _ap(ebT, off + 64, [[128, 8], [8, 8], [1, 8]])
                    zr_ap = _ap(natbf, off + 8, [[128, 8], [16, 8], [1, 8]])
                    tmp = sbp.tile([128, 1024], BF, tag="tmp", bufs=2)
                    tmp_ap = _ap(tmp, 0, [[128, 8], [8, 8], [1, 8]])
                    nc.vector.tensor_mul(out=tmp_ap, in0=e_ap, in1=zr_ap)
                    nc.vector.tensor_add(out=zr_ap, in0=tmp_ap, in1=b_ap)

                # ---- store with cast bf16 -> fp32 (SWDGE)
                out_dma = nc.gpsimd.dma_start(
                    out=o_d[r0:r0 + ROWS_PER_MACRO, :].rearrange(
                        "(p c) f -> p c f", p=128, c=128
                    ),
                    in_=natbf.rearrange("p (c f) -> p c f", c=128, f=16),
                )
                if m >= MACROS - 3:
                    tail_dmas.append(out_dma)

            flush = sbp.tile([128, 1], FP, tag="wu")
            fl = nc.vector.tensor_copy(out=flush, in_=biases[0])
            for dma in tail_dmas:
                _add_dep_helper(fl.ins, dma.ins, sync=True,
                                reason="drain tail out-DMAs before kernel end")

    nc.finalize()
    return nc


_NC_CACHE = None


def kernel(z, ws_logs, bs_logs, ws_b, bs_b):
    global _NC_CACHE, LAST_RESULTS
    z = np.asarray(z, np.float32)
    assert z.shape == (BATCH, 16)
    consts, wmat_bf = _build_consts(ws_logs, bs_logs, ws_b, bs_b)

    if _NC_CACHE is None:
        _NC_CACHE = _build_nc()
    nc = _NC_CACHE

    in_maps = []
    for s in STARTS:
        zp = np.zeros((R + PAD_ROWS, 16), np.float32)
        zp[:R] = z[s:s + R]
        in_maps.append({"z": zp, "consts": consts, "wmat": wmat_bf})
    trace = bool(os.environ.get("AFFINE_TRACE"))
    res = run_bass_kernel_spmd(nc, in_maps, core_ids=list(range(N_CORES)), trace=trace)
    LAST_RESULTS = res

    out = np.empty((BATCH, 16), np.float32)
    for c in range(N_CORES):
        out[STARTS[c]:STARTS[c] + R] = res.results[c]["out"][:R]
    return out


# revision 4
# speedup vs baseline: 1.5806x; 1.1170x over previous
"""AffineCoupling TRN2 kernel (v6).

Computes, for z [4_000_000, 16] fp32:
    zl = z[:, :8]; zr = z[:, 8:]
    log_s = MLP_logs(zl); b = MLP_b(zl)        (5 layers, LeakyReLU(0.01) between)
    out = concat([zl, yr]), yr = exp(log_s) * zr + b

Strategy (pure data parallel over 8 NeuronCores, ~508k rows each):
 - Contiguous DMA: core slice split into 31 macros of 16384 rows. natbf
   [128, 2048] bf16 holds 128 rows/partition (nat[p, c*16+f] = row p*128+c),
   loaded by ONE SWDGE cast-DMA (fp32 HBM -> bf16 SBUF, 8KB contiguous HBM
   per partition) and stored back by one SWDGE cast-DMA (bf16 -> fp32).
   The whole pipeline is bf16 (zl passthrough in bf16: ~1.3e-3 rel err,
   tolerance is 2e-2).
 - fwdT: 16 PE transpose-mode ops [128,128] -> x0ps bf16 PSUM (feature-major
   X layout: partition g*16+f, 8 groups of 16 feats); DVE 2x copy -> x0.
 - MLP: both branches fused in 16-wide groups (block-diagonal bf16 lhsT,
   same wmat as before); per layer 4 MMs N=512 -> h fp32 PSUM [128, 2048],
   one ACT Prelu (bias via per-partition operand) -> bf16 SBUF.
 - L5 -> hp5 [128, 2048] fp32 (e at partitions 0:64 as g*8+o, b at 64:128);
   ACT Exp (+bias) and DVE tensor_scalar_add assemble eb bf16.
 - backT: 16 transpose-mode ops -> ebT bf16 PSUM; combine in place:
   natbf_zr = e*zr + b via 2x (mul into tmp, add back), u-batched APs.
 - PSUM: x0ps(2) + h(4) + ebT(2) = 8 banks, single-buffered per tag;
   cross-macro overlap comes from fwdT/backT of adjacent macros.
"""
import os
import sys

sys.path.insert(0, "/opt/trn_rl_repo")
if "/root/.axon_site/_ro/trn_rl_repo" not in sys.path:
    sys.path.append("/root/.axon_site/_ro/trn_rl_repo")

import numpy as np

import concourse.bacc as bacc
import concourse.bass as bass
import concourse.tile as tile
from concourse import mybir
from concourse.bass import _add_dep_helper
from concourse.bass_utils import run_bass_kernel_spmd

FP = mybir.dt.float32
BF = mybir.dt.bfloat16

N_CORES = 8
BATCH = 4_000_000
ROWS_PER_MACRO = 16_384            # [128, 2048] bf16 nat tile, 128 rows/part
MACROS = 31
R = ROWS_PER_MACRO * MACROS        # 507,904 rows per core
PAD_ROWS = ROWS_PER_MACRO
CHUNKS = 4                         # 4096 rows each

STEP = 498_688
STARTS = [c * STEP for c in range(N_CORES - 1)] + [BATCH - R]

C_BIAS = 128
C_TOTAL = 133

LAST_RESULTS = None


def _build_consts(ws_logs, bs_logs, ws_b, bs_b):
    import ml_dtypes

    ws_logs = [np.asarray(w, np.float32) for w in ws_logs]
    bs_logs = [np.asarray(b, np.float32) for b in bs_logs]
    ws_b = [np.asarray(w, np.float32) for w in ws_b]
    bs_b = [np.asarray(b, np.float32) for b in bs_b]

    consts = np.zeros((128, C_TOTAL), np.float32)
    consts[:, 0:128] = np.eye(128, dtype=np.float32)
    for k in range(4):
        cat = np.concatenate([bs_logs[k], bs_b[k]])    # [16]
        consts[:, C_BIAS + k] = np.tile(cat, 8)
    consts[:, C_BIAS + 4] = np.concatenate(
        [np.tile(bs_logs[4], 8), np.tile(bs_b[4], 8)]
    )

    # bf16 stationary matrices, lhsT k at cols [k*128, (k+1)*128)
    wmat = np.zeros((128, 5 * 128), np.float32)
    w1cat = np.vstack([ws_logs[0], ws_b[0]])           # [16, 8]
    for g in range(8):
        wmat[g * 16:g * 16 + 8, g * 16:(g + 1) * 16] = w1cat.T
    for k in (1, 2, 3):
        wk = np.zeros((16, 16), np.float32)
        wk[0:8, 0:8] = ws_logs[k]
        wk[8:16, 8:16] = ws_b[k]
        for g in range(8):
            wmat[g * 16:(g + 1) * 16, k * 128 + g * 16:k * 128 + (g + 1) * 16] = wk.T
    for g in range(8):
        wmat[g * 16:g * 16 + 8, 4 * 128 + g * 8:4 * 128 + (g + 1) * 8] = ws_logs[4].T
        wmat[g * 16 + 8:(g + 1) * 16,
             4 * 128 + 64 + g * 8:4 * 128 + 64 + (g + 1) * 8] = ws_b[4].T
    wmat = np.concatenate([wmat, np.eye(128, dtype=np.float32)], axis=1)
    wmat_bf = wmat.astype(ml_dtypes.bfloat16)
    return consts, wmat_bf


def _ap(t, offset, dims):
    return bass.AP(tensor=t.tensor, offset=t.offset + offset, ap=[t.ap[0]] + dims)


def _build_nc():
    nc = bacc.Bacc()
    z_d = nc.declare_dram_parameter("z", [R + PAD_ROWS, 16], FP, isOutput=False)
    c_d = nc.declare_dram_parameter("consts", [128, C_TOTAL], FP, isOutput=False)
    w_d = nc.declare_dram_parameter("wmat", [128, 6 * 128], BF, isOutput=False)
    o_d = nc.declare_dram_parameter("out", [R + PAD_ROWS, 16], FP, isOutput=True)

    with tile.TileContext(nc) as tc:
        with (
            tc.tile_pool(name="consts", bufs=1) as cp,
            tc.tile_pool(name="nat", bufs=1) as natp,
            tc.tile_pool(name="sb", bufs=1) as sbp,
            tc.tile_pool(name="ps", bufs=1, space="PSUM") as psp,
        ):
            consts = cp.tile([128, C_TOTAL], FP)
            nc.sync.dma_start(out=consts, in_=c_d[:, :])
            wmat = cp.tile([128, 6 * 128], BF)
            nc.sync.dma_start(out=wmat, in_=w_d[:, :])
            identbf = wmat[:, 5 * 128:6 * 128]
            lhsT = [wmat[:, k * 128:(k + 1) * 128] for k in range(5)]
            biases = [consts[:, C_BIAS + k:C_BIAS + k + 1] for k in range(5)]

            # warm-up scalar/vector vector clocks on the const DMAs
            wu1 = sbp.tile([128, 1], FP, tag="wu")
            nc.scalar.copy(out=wu1, in_=biases[0])
            wu2 = sbp.tile([128, 1], FP, tag="wu")
            nc.vector.tensor_copy(out=wu2, in_=biases[0])

            natbfs = {}

            def load(m):
                if m >= MACROS:
                    return
                r0 = m * ROWS_PER_MACRO
                natbf = natp.tile([128, 2048], BF, tag="nat", bufs=4)
                nc.gpsimd.dma_start(
                    out=natbf.rearrange("p (c f) -> p c f", c=128, f=16),
                    in_=z_d[r0:r0 + ROWS_PER_MACRO, :].rearrange(
                        "(p c) f -> p c f", p=128, c=128
                    ),
                )
                natbfs[m] = natbf

            def fwdT_quarter(m, q, x0ps):
                for u in range(q * 4, q * 4 + 4):
                    nc.tensor.transpose(
                        x0ps[:, u * 128:(u + 1) * 128],
                        natbfs[m][:, u * 128:(u + 1) * 128],
                        identbf,
                    )

            def mlp_mms(lhsT_k, h_in):
                hps = psp.tile([128, 2048], FP, tag="h", bufs=1)
                for n in range(CHUNKS):
                    nc.tensor.matmul(
                        hps[:, n * 512:(n + 1) * 512],
                        lhsT_k, h_in[:, n * 512:(n + 1) * 512],
                        start=True, stop=True,
                    )
                return hps

            def prelu(hps, k):
                hb = sbp.tile([128, 2048], BF, tag=f"h{k}", bufs=2)
                nc.scalar.activation(
                    out=hb, in_=hps,
                    func=mybir.ActivationFunctionType.Prelu,
                    bias=biases[k], scale=1.0, alpha=0.01,
                )
                return hb

            # Software pipeline, skewed by one macro:
            #   iteration i runs layers 2..5 + tail of macro i, interleaving
            #   fwd transposes / x0-evac / L1 of macro i+1 into the PE gaps.
            tail_dmas = []
            load(0)
            load(1)
            load(2)
            x0ps = psp.tile([128, 2048], BF, tag="x0ps", bufs=1)
            for q in range(4):
                fwdT_quarter(0, q, x0ps)
            x0 = sbp.tile([128, 2048], BF, tag="x0", bufs=2)
            nc.vector.tensor_copy(out=x0, in_=x0ps)
            h = prelu(mlp_mms(lhsT[0], x0), 0)

            for m in range(MACROS):
                nxt = m + 1 < MACROS
                if nxt:
                    x0ps = psp.tile([128, 2048], BF, tag="x0ps", bufs=1)
                # layers 2..4 with fwd-transpose quarters of macro m+1 in gaps
                for k in (1, 2, 3):
                    hps = mlp_mms(lhsT[k], h)
                    if nxt:
                        fwdT_quarter(m + 1, k - 1, x0ps)
                    h = prelu(hps, k)
                # L5
                hp5 = mlp_mms(lhsT[4], h)
                if nxt:
                    fwdT_quarter(m + 1, 3, x0ps)
                eb = sbp.tile([128, 2048], BF, tag="eb", bufs=2)
                nc.scalar.activation(
                    out=eb[0:64, :], in_=hp5[0:64, :],
                    func=mybir.ActivationFunctionType.Exp,
                    bias=biases[4][0:64, :], scale=1.0,
                )
                nc.vector.tensor_scalar_add(
                    out=eb[64:128, :], in0=hp5[64:128, :],
                    scalar1=biases[4][64:128, :],
                )
                if nxt:
                    x0 = sbp.tile([128, 2048], BF, tag="x0", bufs=2)
                    nc.vector.tensor_copy(out=x0, in_=x0ps)
                    # L1 of macro m+1 fills PE while ACT runs Exp/Prelus
                    h = prelu(mlp_mms(lhsT[0], x0), 0)

                # ---- back transposes -> ebT (bf16 PSUM)
                ebT = psp.tile([128, 2048], BF, tag="ebT", bufs=1)
                for u in range(16):
                    nc.tensor.transpose(
                        ebT[:, u * 128:(u + 1) * 128],
                        eb[:, u * 128:(u + 1) * 128],
                        identbf,
                    )

                # ---- combine in place: natbf_zr = e*zr + b  (u-batched halves)
                natbf = natbfs[m]
                for half in range(2):
                    off = half * 1024
                    e_ap = _ap(ebT, off, [[128, 8], [8, 8], [1, 8]])
                    b_ap = _ap(ebT, off + 64, [[128, 8], [8, 8], [1, 8]])
                    zr_ap = _ap(natbf, off + 8, [[128, 8], [16, 8], [1, 8]])
                    tmp = sbp.tile([128, 1024], BF, tag="tmp", bufs=2)
                    tmp_ap = _ap(tmp, 0, [[128, 8], [8, 8], [1, 8]])
                    nc.vector.tensor_mul(out=tmp_ap, in0=e_ap, in1=zr_ap)
                    nc.vector.tensor_add(out=zr_ap, in0=tmp_ap, in1=b_ap)

                # ---- store with cast bf16 -> fp32 (SWDGE)
                r0 = m * ROWS_PER_MACRO
                out_dma = nc.gpsimd.dma_start(
                    out=o_d[r0:r0 + ROWS_PER_MACRO, :].rearrange(
                        "(p c) f -> p c f", p=128, c=128
                    ),
                    in_=natbf.rearrange("p (c f) -> p c f", c=128, f=16),
                )
                del natbfs[m]
                load(m + 3)
                if m >= MACROS - 4:
                    tail_dmas.append(out_dma)

            flush = sbp.tile([128, 1], FP, tag="wu")
            fl = nc.vector.tensor_copy(out=flush, in_=biases[0])
            for dma in tail_dmas:
                _add_dep_helper(fl.ins, dma.ins, sync=True,
                                reason="drain tail out-DMAs before kernel end")

    nc.finalize()
    return nc


_NC_CACHE = None


def kernel(z, ws_logs, bs_logs, ws_b, bs_b):
    global _NC_CACHE, LAST_RESULTS
    z = np.asarray(z, np.float32)
    assert z.shape == (BATCH, 16)
    consts, wmat_bf = _build_consts(ws_logs, bs_logs, ws_b, bs_b)

    if _NC_CACHE is None:
        _NC_CACHE = _build_nc()
    nc = _NC_CACHE

    in_maps = []
    for s in STARTS:
        zp = np.zeros((R + PAD_ROWS, 16), np.float32)
        zp[:R] = z[s:s + R]
        in_maps.append({"z": zp, "consts": consts, "wmat": wmat_bf})
    trace = bool(os.environ.get("AFFINE_TRACE"))
    res = run_bass_kernel_spmd(nc, in_maps, core_ids=list(range(N_CORES)), trace=trace)
    LAST_RESULTS = res

    out = np.empty((BATCH, 16), np.float32)
    for c in range(N_CORES):
        out[STARTS[c]:STARTS[c] + R] = res.results[c]["out"][:R]
    return out


# revision 5
# speedup vs baseline: 2.4207x; 1.5316x over previous
"""AffineCoupling TRN2 kernel (v6).

Computes, for z [4_000_000, 16] fp32:
    zl = z[:, :8]; zr = z[:, 8:]
    log_s = MLP_logs(zl); b = MLP_b(zl)        (5 layers, LeakyReLU(0.01) between)
    out = concat([zl, yr]), yr = exp(log_s) * zr + b

Strategy (pure data parallel over 8 NeuronCores, ~508k rows each):
 - Contiguous DMA: core slice split into 31 macros of 16384 rows. natbf
   [128, 2048] bf16 holds 128 rows/partition (nat[p, c*16+f] = row p*128+c),
   loaded by ONE SWDGE cast-DMA (fp32 HBM -> bf16 SBUF, 8KB contiguous HBM
   per partition) and stored back by one SWDGE cast-DMA (bf16 -> fp32).
   The whole pipeline is bf16 (zl passthrough in bf16: ~1.3e-3 rel err,
   tolerance is 2e-2).
 - fwdT: 16 PE transpose-mode ops [128,128] -> x0ps bf16 PSUM (feature-major
   X layout: partition g*16+f, 8 groups of 16 feats); DVE 2x copy -> x0.
 - MLP: both branches fused in 16-wide groups (block-diagonal bf16 lhsT,
   same wmat as before); per layer 4 MMs N=512 -> h fp32 PSUM [128, 2048],
   one ACT Prelu (bias via per-partition operand) -> bf16 SBUF.
 - L5 -> hp5 [128, 2048] fp32 (e at partitions 0:64 as g*8+o, b at 64:128);
   ACT Exp (+bias) and DVE tensor_scalar_add assemble eb bf16.
 - backT: 16 transpose-mode ops -> ebT bf16 PSUM; combine in place:
   natbf_zr = e*zr + b via 2x (mul into tmp, add back), u-batched APs.
 - PSUM: x0ps(2) + h(4) + ebT(2) = 8 banks, single-buffered per tag;
   cross-macro overlap comes from fwdT/backT of adjacent macros.
"""
import os
import sys

sys.path.insert(0, "/opt/trn_rl_repo")
if "/root/.axon_site/_ro/trn_rl_repo" not in sys.path:
    sys.path.append("/root/.axon_site/_ro/trn_rl_repo")

import numpy as np

import concourse.bacc as bacc
import concourse.bass as bass
import concourse.tile as tile
from concourse import mybir
from concourse.bass import _add_dep_helper
from concourse.bass_utils import run_bass_kernel_spmd

FP = mybir.dt.float32
BF = mybir.dt.bfloat16

N_CORES = 8
BATCH = 4_000_000
ROWS_PER_MACRO = 16_384            # [128, 2048] bf16 nat tile, 128 rows/part
MACROS = 31
R = ROWS_PER_MACRO * MACROS        # 507,904 rows per core
PAD_ROWS = ROWS_PER_MACRO
CHUNKS = 4                         # 4096 rows each

STEP = 498_688
STARTS = [c * STEP for c in range(N_CORES - 1)] + [BATCH - R]

C_BIAS = 128
C_TOTAL = 133

LAST_RESULTS = None


def _build_consts(ws_logs, bs_logs, ws_b, bs_b):
    import ml_dtypes

    ws_logs = [np.asarray(w, np.float32) for w in ws_logs]
    bs_logs = [np.asarray(b, np.float32) for b in bs_logs]
    ws_b = [np.asarray(w, np.float32) for w in ws_b]
    bs_b = [np.asarray(b, np.float32) for b in bs_b]

    consts = np.zeros((128, C_TOTAL), np.float32)
    consts[:, 0:128] = np.eye(128, dtype=np.float32)
    for k in range(4):
        cat = np.concatenate([bs_logs[k], bs_b[k]])    # [16]
        consts[:, C_BIAS + k] = np.tile(cat, 8)
    consts[:, C_BIAS + 4] = np.concatenate(
        [np.tile(bs_logs[4], 8), np.tile(bs_b[4], 8)]
    )

    # bf16 stationary matrices, lhsT k at cols [k*128, (k+1)*128)
    wmat = np.zeros((128, 5 * 128), np.float32)
    w1cat = np.vstack([ws_logs[0], ws_b[0]])           # [16, 8]
    for g in range(8):
        wmat[g * 16:g * 16 + 8, g * 16:(g + 1) * 16] = w1cat.T
    for k in (1, 2, 3):
        wk = np.zeros((16, 16), np.float32)
        wk[0:8, 0:8] = ws_logs[k]
        wk[8:16, 8:16] = ws_b[k]
        for g in range(8):
            wmat[g * 16:(g + 1) * 16, k * 128 + g * 16:k * 128 + (g + 1) * 16] = wk.T
    for g in range(8):
        wmat[g * 16:g * 16 + 8, 4 * 128 + g * 8:4 * 128 + (g + 1) * 8] = ws_logs[4].T
        wmat[g * 16 + 8:(g + 1) * 16,
             4 * 128 + 64 + g * 8:4 * 128 + 64 + (g + 1) * 8] = ws_b[4].T
    wmat = np.concatenate([wmat, np.eye(128, dtype=np.float32)], axis=1)
    wmat_bf = wmat.astype(ml_dtypes.bfloat16)
    return consts, wmat_bf


def _ap(t, offset, dims):
    return bass.AP(tensor=t.tensor, offset=t.offset + offset, ap=[t.ap[0]] + dims)


def _build_nc():
    nc = bacc.Bacc()
    z_d = nc.declare_dram_parameter("z", [R + PAD_ROWS, 16], FP, isOutput=False)
    c_d = nc.declare_dram_parameter("consts", [128, C_TOTAL], FP, isOutput=False)
    w_d = nc.declare_dram_parameter("wmat", [128, 6 * 128], BF, isOutput=False)
    o_d = nc.declare_dram_parameter("out", [R + PAD_ROWS, 16], FP, isOutput=True)

    with tile.TileContext(nc) as tc:
        with (
            tc.tile_pool(name="consts", bufs=1) as cp,
            tc.tile_pool(name="nat", bufs=1) as natp,
            tc.tile_pool(name="sb", bufs=1) as sbp,
            tc.tile_pool(name="ps", bufs=1, space="PSUM") as psp,
        ):
            consts = cp.tile([128, C_TOTAL], FP)
            nc.sync.dma_start(out=consts, in_=c_d[:, :])
            wmat = cp.tile([128, 6 * 128], BF)
            nc.sync.dma_start(out=wmat, in_=w_d[:, :])
            identbf = wmat[:, 5 * 128:6 * 128]
            lhsT = [wmat[:, k * 128:(k + 1) * 128] for k in range(5)]
            biases = [consts[:, C_BIAS + k:C_BIAS + k + 1] for k in range(5)]

            # warm-up scalar/vector vector clocks on the const DMAs
            wu1 = sbp.tile([128, 1], FP, tag="wu")
            nc.scalar.copy(out=wu1, in_=biases[0])
            wu2 = sbp.tile([128, 1], FP, tag="wu")
            nc.vector.tensor_copy(out=wu2, in_=biases[0])

            natbfs = {}

            def load(m):
                if m >= MACROS:
                    return
                r0 = m * ROWS_PER_MACRO
                natbf = natp.tile([128, 2048], BF, tag="nat", bufs=4)
                nc.gpsimd.dma_start(
                    out=natbf.rearrange("p (c f) -> p c f", c=128, f=16),
                    in_=z_d[r0:r0 + ROWS_PER_MACRO, :].rearrange(
                        "(p c) f -> p c f", p=128, c=128
                    ),
                )
                natbfs[m] = natbf

            def fwdT_quarter(m, q, x0ps):
                for u in range(q * 4, q * 4 + 4):
                    nc.tensor.transpose(
                        x0ps[:, u * 128:(u + 1) * 128],
                        natbfs[m][:, u * 128:(u + 1) * 128],
                        identbf,
                    )

            def half_mms(lhsT_k, h_in, half, tag):
                hps = psp.tile([128, 1024], FP, tag=tag, bufs=1)
                for n in range(2):
                    src = h_in[:, half * 1024 + n * 512:half * 1024 + (n + 1) * 512]
                    nc.tensor.matmul(hps[:, n * 512:(n + 1) * 512],
                                     lhsT_k, src, start=True, stop=True)
                return hps

            def half_prelu(hps, k, hb, half):
                nc.scalar.activation(
                    out=hb[:, half * 1024:(half + 1) * 1024], in_=hps,
                    func=mybir.ActivationFunctionType.Prelu,
                    bias=biases[k], scale=1.0, alpha=0.01,
                )

            def mlp(x0, eb):
                """Full 5-layer MLP with A/B column-half ping-pong."""
                h = x0
                for k in range(4):
                    hb = sbp.tile([128, 2048], BF, tag=f"h{k}", bufs=2)
                    hA = half_mms(lhsT[k], h, 0, "hA")
                    hB = half_mms(lhsT[k], h, 1, "hB")
                    half_prelu(hA, k, hb, 0)
                    half_prelu(hB, k, hb, 1)
                    h = hb
                for half, tag in ((0, "hA"), (1, "hB")):
                    hp5 = half_mms(lhsT[4], h, half, tag)
                    nc.scalar.activation(
                        out=eb[0:64, half * 1024:(half + 1) * 1024],
                        in_=hp5[0:64, :],
                        func=mybir.ActivationFunctionType.Exp,
                        bias=biases[4][0:64, :], scale=1.0,
                    )
                    nc.vector.tensor_scalar_add(
                        out=eb[64:128, half * 1024:(half + 1) * 1024],
                        in0=hp5[64:128, :],
                        scalar1=biases[4][64:128, :],
                    )

            # Software pipeline, skewed by one macro: iteration m runs the MLP
            # of macro m (A/B halves ping-ponging PE vs ACT), with macro m+1's
            # fwd transposes and macro m's back transposes filling PE slack.
            tail_dmas = []
            load(0)
            load(1)
            load(2)
            x0ps = psp.tile([128, 2048], BF, tag="x0ps", bufs=1)
            for q in range(4):
                fwdT_quarter(0, q, x0ps)
            x0 = sbp.tile([128, 2048], BF, tag="x0", bufs=2)
            nc.vector.tensor_copy(out=x0, in_=x0ps)

            for m in range(MACROS):
                nxt = m + 1 < MACROS
                eb = sbp.tile([128, 2048], BF, tag="eb", bufs=2)
                mlp(x0, eb)
                if nxt:
                    x0ps = psp.tile([128, 2048], BF, tag="x0ps", bufs=1)
                    for q in range(4):
                        fwdT_quarter(m + 1, q, x0ps)
                    x0 = sbp.tile([128, 2048], BF, tag="x0", bufs=2)
                    nc.vector.tensor_copy(out=x0, in_=x0ps)

                # ---- back transposes -> ebT (bf16 PSUM)
                ebT = psp.tile([128, 2048], BF, tag="ebT", bufs=1)
                for u in range(16):
                    nc.tensor.transpose(
                        ebT[:, u * 128:(u + 1) * 128],
                        eb[:, u * 128:(u + 1) * 128],
                        identbf,
                    )

                # ---- combine in place: natbf_zr = e*zr + b  (u-batched halves)
                natbf = natbfs[m]
                for half in range(2):
                    off = half * 1024
                    e_ap = _ap(ebT, off, [[128, 8], [8, 8], [1, 8]])
                    b_ap = _ap(ebT, off + 64, [[128, 8], [8, 8], [1, 8]])
                    zr_ap = _ap(natbf, off + 8, [[128, 8], [16, 8], [1, 8]])
                    tmp = sbp.tile([128, 1024], BF, tag="tmp", bufs=2)
                    tmp_ap = _ap(tmp, 0, [[128, 8], [8, 8], [1, 8]])
                    nc.vector.tensor_mul(out=tmp_ap, in0=e_ap, in1=zr_ap)
                    nc.vector.tensor_add(out=zr_ap, in0=tmp_ap, in1=b_ap)

                # ---- store with cast bf16 -> fp32 (SWDGE)
                r0 = m * ROWS_PER_MACRO
                out_dma = nc.gpsimd.dma_start(
                    out=o_d[r0:r0 + ROWS_PER_MACRO, :].rearrange(
                        "(p c) f -> p c f", p=128, c=128
                    ),
                    in_=natbf.rearrange("p (c f) -> p c f", c=128, f=16),
                )
                del natbfs[m]
                load(m + 3)
                if m >= MACROS - 4:
                    tail_dmas.append(out_dma)

            flush = sbp.tile([128, 1], FP, tag="wu")
            fl = nc.vector.tensor_copy(out=flush, in_=biases[0])
            for dma in tail_dmas:
                _add_dep_helper(fl.ins, dma.ins, sync=True,
                                reason="drain tail out-DMAs before kernel end")

    nc.finalize()
    return nc


_NC_CACHE = None


def kernel(z, ws_logs, bs_logs, ws_b, bs_b):
    global _NC_CACHE, LAST_RESULTS
    z = np.asarray(z, np.float32)
    assert z.shape == (BATCH, 16)
    consts, wmat_bf = _build_consts(ws_logs, bs_logs, ws_b, bs_b)

    if _NC_CACHE is None:
        _NC_CACHE = _build_nc()
    nc = _NC_CACHE

    in_maps = []
    for s in STARTS:
        zp = np.zeros((R + PAD_ROWS, 16), np.float32)
        zp[:R] = z[s:s + R]
        in_maps.append({"z": zp, "consts": consts, "wmat": wmat_bf})
    trace = bool(os.environ.get("AFFINE_TRACE"))
    res = run_bass_kernel_spmd(nc, in_maps, core_ids=list(range(N_CORES)), trace=trace)
    LAST_RESULTS = res

    out = np.empty((BATCH, 16), np.float32)
    for c in range(N_CORES):
        out[STARTS[c]:STARTS[c] + R] = res.results[c]["out"][:R]
    return out
